# revision 47
# baseline (speedup 1.0000x reference)
"""HAN layer (4 metapaths x 2-layer mean-RGCN + metapath attention) on 8 trn2 cores.

Sharding: cores (2i, 2i+1) handle metapath i, splitting 128-dst blocks by
PARITY (even blocks -> core 2i, odd -> 2i+1) for BOTH layers. With node rows
stored in parity-permuted order (perm(j) = side*ng1*128 + (blk>>1)*128 + pos),
layer-2's edge grid is exactly the first ng2 groups of layer-1's grid — the
same bytes serve both layers, and both gather tables (x0perm, x1_full) share
the perm layout so gather indices coincide. Attention: score AllGather +
ReduceScatter over {0,2,4,6} / {1,3,5,7}; host interleaves blocks back.

Device algorithm per layer (linearity: segment_sum(x[src]) @ Wm): edges are
host-sorted into 128-dst groups; an indirect DMA gathers x[src] rows for a
group; per 128-edge chunk a selector eq[e,d] = (dl[e]==d)*rec[e] is built on
DVE and matmul-accumulated on PE into meanT = (segment_mean)^T in PSUM; two
dense matmuls + fused ReLU produce the group's 128 output rows, written
contiguously (no scatter anywhere).

Host->device transfer dominates (narrow tunnel, ~44MB/s, no compression, big
per-buffer fixed cost): E is int8-quantized (scale folded into L1 weights),
compacted to used rows, sharded 1/8 + device AllGather; x0perm = E[eids] is
materialized on device from a shipped index list; grids are 4B/slot (u16
idx-lo + u8 [idx-hi<<6|deg] + u8 dst-local) unpacked on DVE; 6 input buffers
total; all compute bf16 with f32 PSUM.
"""

import math
import numpy as np

try:
    # run_bass_kernel_spmd re-jits an identical XLA wrapper on every call;
    # the persistent compilation cache makes those re-jits near-free.
    import jax as _jax
    _jax.config.update("jax_compilation_cache_dir", "/tmp/jax_cc")
    _jax.config.update("jax_persistent_cache_min_entry_size_bytes", -1)
    _jax.config.update("jax_persistent_cache_min_compile_time_secs", 0.0)
except Exception:
    pass

import concourse.bass as bass
import concourse.bacc as bacc
import concourse.mybir as mybir
from concourse.tile import TileContext
from concourse.bass_utils import run_bass_kernel_spmd

F32 = mybir.dt.float32
BF16 = mybir.dt.bfloat16
I32 = mybir.dt.int32
I8 = mybir.dt.int8
U16 = mybir.dt.uint16
U8 = mybir.dt.uint8

N_CORES = 8
BF = 4     # output groups batched per store DMA
CH = 16    # groups per grid-load DMA


def _np_bf16():
    import ml_dtypes
    return ml_dtypes.bfloat16


# ----------------------------------------------------------------- host prep

def _build_grids(idxs, gidx, dl, degv, ng, nb):
    """Packed grid for dst-sorted edges with group ids gidx (non-decreasing)
    and dst-local ids dl. Slot j = p*nb + b of group g lands at partition p,
    column g*nb + b. Ships u16 idx-low + u8 (idx-high<<6 | deg) + u8 dl.
    Empty slots: dl=128 (selector row all-zero), pk=1 (finite reciprocal)."""
    assert degv.size == 0 or degv.max() <= 63
    assert idxs.size == 0 or idxs.max() < (1 << 18)
    starts = np.searchsorted(gidx, np.arange(ng))
    slot = np.arange(len(gidx)) - starts[gidx]
    p = slot & 127
    b = slot >> 7
    col = gidx * nb + b
    S = nb * ng
    lo16 = np.zeros((128, S), np.uint16)
    pk8 = np.ones((128, S), np.uint8)
    dl8 = np.full((128, S), 128, np.uint8)
    lo16[p, col] = (idxs & 0xFFFF).astype(np.uint16)
    pk8[p, col] = (((idxs >> 16) << 6) | degv).astype(np.uint8)
    dl8[p, col] = dl.astype(np.uint8)
    return lo16, pk8, dl8


# ------------------------------------------------------------- device build

def _emit_layer(nc, tc, pools, table, table_i8, n_hi, gall, S1,
                basep_t, wm_t, wr_t, ng, nb, iota_t, ident_t,
                out_dram, rows_total, hook=None):
    # gall u8 sections: [lo_b0 | lo_b1 | pk | dl] at offsets 0,S1,2S1,3S1
    sb, sbg, psum, sbeq = pools
    stage = None
    for g in range(ng):
        if g % CH == 0:
            w = min(CH, ng - g)
            l0t = sbg.tile([128, nb * w], U8, tag="l0t")
            nc.sync.dma_start(out=l0t[:], in_=gall[:, g * nb:(g + w) * nb])
            l1t = sbg.tile([128, nb * w], U8, tag="l1t")
            nc.sync.dma_start(
                out=l1t[:], in_=gall[:, S1 + g * nb:S1 + (g + w) * nb])
            pkt = sbg.tile([128, nb * w], U8, tag="pkt")
            nc.sync.dma_start(
                out=pkt[:], in_=gall[:, 2 * S1 + g * nb:2 * S1 + (g + w) * nb])
            dlt8 = sbg.tile([128, nb * w], U8, tag="dlt8")
            nc.sync.dma_start(
                out=dlt8[:], in_=gall[:, 3 * S1 + g * nb:3 * S1 + (g + w) * nb])
            # xd rows are h*nperm + 128*(g+j) + p in both layers' tables:
            # build on DVE from the shipped per-partition base column
            idxdf = sbg.tile([128, w], F32, tag="idxdf")
            nc.vector.tensor_scalar(out=idxdf[:], in0=iota_t[:, :w],
                                    scalar1=128.0, scalar2=basep_t[:, 0:1],
                                    op0=mybir.AluOpType.mult,
                                    op1=mybir.AluOpType.add)
            idxdt = sbg.tile([128, w], I32, tag="idxdt")
            nc.vector.tensor_scalar(out=idxdt[:], in0=idxdf[:],
                                    scalar1=float(128 * g), scalar2=None,
                                    op0=mybir.AluOpType.add)
            # unpack pk = hi<<6 | deg without mod: 64*hi via is_ge steps,
            # idx = lo + 65536*hi (exact in f32: < 2^24), rec = 1/deg
            pkf = sbg.tile([128, nb * w], F32, tag="pkf")
            nc.vector.tensor_copy(out=pkf[:], in_=pkt[:])
            hi64 = sbg.tile([128, nb * w], F32, tag="hi64")
            nc.vector.tensor_scalar(out=hi64[:], in0=pkf[:], scalar1=64.0,
                                    scalar2=64.0, op0=mybir.AluOpType.is_ge,
                                    op1=mybir.AluOpType.mult)
            for k in range(1, n_hi):
                hpart = sbg.tile([128, nb * w], F32, tag="hpart")
                nc.vector.tensor_scalar(
                    out=hpart[:], in0=pkf[:], scalar1=64.0 * (k + 1),
                    scalar2=64.0, op0=mybir.AluOpType.is_ge,
                    op1=mybir.AluOpType.mult)
                nc.vector.tensor_tensor(out=hi64[:], in0=hi64[:], in1=hpart[:],
                                        op=mybir.AluOpType.add)
            dgf = sbg.tile([128, nb * w], F32, tag="dgf")
            nc.vector.tensor_tensor(out=dgf[:], in0=pkf[:], in1=hi64[:],
                                    op=mybir.AluOpType.subtract)
            lof = sbg.tile([128, nb * w], F32, tag="lof")
            nc.vector.tensor_copy(out=lof[:], in_=l0t[:])
            l1f = sbg.tile([128, nb * w], F32, tag="l1f")
            nc.vector.tensor_scalar(out=l1f[:], in0=l1t[:], scalar1=256.0,
                                    scalar2=None, op0=mybir.AluOpType.mult)
            nc.vector.tensor_tensor(out=lof[:], in0=lof[:], in1=l1f[:],
                                    op=mybir.AluOpType.add)
            nc.vector.tensor_scalar(out=hi64[:], in0=hi64[:], scalar1=1024.0,
                                    scalar2=None, op0=mybir.AluOpType.mult)
            idxt = sbg.tile([128, nb * w], I32, tag="idxt")
            nc.vector.tensor_tensor(out=idxt[:], in0=hi64[:], in1=lof[:],
                                    op=mybir.AluOpType.add)
            dlf = sbg.tile([128, nb * w], F32, tag="dlf")
            nc.vector.tensor_copy(out=dlf[:], in_=dlt8[:])
            recf = sbg.tile([128, nb * w], F32, tag="recf")
            nc.vector.reciprocal(out=recf[:], in_=dgf[:])
        o = (g % CH) * nb

        if table_i8:
            msgs_raw = sb.tile([128, nb * 128], I8, tag="msgs_raw")
        else:
            msgs_raw = sb.tile([128, nb * 128], BF16, tag="msgs")
        for b in range(nb):
            nc.gpsimd.indirect_dma_start(
                out=msgs_raw[:, b * 128:(b + 1) * 128], out_offset=None,
                in_=table[:],
                in_offset=bass.IndirectOffsetOnAxis(
                    ap=idxt[:, o + b:o + b + 1], axis=0))
        if table_i8:
            msgs = sb.tile([128, nb * 128], BF16, tag="msgs")
            nc.vector.tensor_copy(out=msgs[:], in_=msgs_raw[:])
        else:
            msgs = msgs_raw

        meant_ps = psum.tile([128, 128], F32, space="PSUM", tag="meant")
        for b in range(nb):
            eq = sbeq.tile([128, 128], BF16, tag="eq")
            nc.vector.tensor_scalar(
                out=eq[:], in0=iota_t[:],
                scalar1=dlf[:, o + b:o + b + 1], scalar2=recf[:, o + b:o + b + 1],
                op0=mybir.AluOpType.is_equal, op1=mybir.AluOpType.mult)
            nc.tensor.matmul(out=meant_ps[:], lhsT=msgs[:, b * 128:(b + 1) * 128],
                             rhs=eq[:], start=(b == 0), stop=(b == nb - 1))
        meant = sb.tile([128, 128], BF16, tag="meant_sb")
        nc.vector.tensor_copy(out=meant[:], in_=meant_ps[:])

        if table_i8:
            xd_raw = sb.tile([128, 128], I8, tag="xd_raw")
        else:
            xd_raw = sb.tile([128, 128], BF16, tag="xd")
        nc.gpsimd.indirect_dma_start(
            out=xd_raw[:], out_offset=None, in_=table[:],
            in_offset=bass.IndirectOffsetOnAxis(
                ap=idxdt[:, g % CH:g % CH + 1], axis=0))
        if table_i8:
            xd = sb.tile([128, 128], BF16, tag="xd")
            nc.vector.tensor_copy(out=xd[:], in_=xd_raw[:])
        else:
            xd = xd_raw
        xdt_ps = psum.tile([128, 128], BF16, space="PSUM", tag="xdt")
        nc.tensor.transpose(out=xdt_ps[:], in_=xd[:], identity=ident_t[:])
        xdt = sb.tile([128, 128], BF16, tag="xdt_sb")
        nc.vector.tensor_copy(out=xdt[:], in_=xdt_ps[:])

        h_ps = psum.tile([128, 128], F32, space="PSUM", tag="hps")
        nc.tensor.matmul(out=h_ps[:], lhsT=meant[:], rhs=wm_t[:],
                         start=True, stop=False)
        nc.tensor.matmul(out=h_ps[:], lhsT=xdt[:], rhs=wr_t[:],
                         start=False, stop=True)

        gb = g % BF
        if gb == 0:
            bw = min(BF, ng - g)
            stage = sb.tile([128, bw * 128], BF16, tag="xn_stage")
        xn = stage[:, gb * 128:(gb + 1) * 128]
        nc.scalar.activation(out=xn, in_=h_ps[:],
                             func=mybir.ActivationFunctionType.Relu)
        if hook is not None:
            hook(g, xn)
        if gb == bw - 1:
            g0 = g - gb
            rows = (gb + 1) * 128
            nc.sync.dma_start(
                out=out_dram[g0 * 128:g0 * 128 + rows, :]
                .rearrange("(a t) f -> t a f", t=128),
                in_=stage[:, :rows].rearrange("p (a f) -> p a f", f=128))


def build_program(etab, ng1, nb1, ng2):
    nc = bacc.Bacc("TRN2", target_bir_lowering=False, debug=False,
                   num_devices=N_CORES)
    nsh = etab // N_CORES
    tb = 2 * ng1                 # total 128-blocks in perm layout
    nperm = ng1 * 128            # rows per parity side
    nrs = (ng2 * 128) // 4       # ReduceScatter rows per rank
    S1 = nb1 * ng1

    def ei(name, shape, dt=F32):
        return nc.dram_tensor(name, shape, dt, kind="ExternalInput")

    # consolidated inputs (per-buffer transfer overhead is large)
    e_sh = ei("e_sh", [nsh, 128], I8)
    # [lo_b0 | lo_b1 | pk | dl | eip_b0 | eip_b1 | eip_hi]
    g_all = ei("g_all", [128, 4 * S1 + 3 * tb], U8)
    cst = ei("cst", [128, 133])                # [iota | sel | basep]
    wts = ei("wts", [128, 6 * 128], BF16)      # [wm1|wr1|wm2|wr2|qs|ident]

    out_part = nc.dram_tensor("out_part", [nrs, 128], BF16,
                              kind="ExternalOutput")

    e_loc = nc.dram_tensor("e_loc", [nsh, 128], I8)
    e_full = nc.dram_tensor("e_full", [nsh * N_CORES, 128], I8)
    x0p = nc.dram_tensor("x0p", [tb * 128, 128], I8)
    x1_half = nc.dram_tensor("x1_half", [nperm, 128], BF16)
    x1_full = nc.dram_tensor("x1_full", [2 * nperm, 128], BF16)
    x2b = nc.dram_tensor("x2b", [ng2 * 128, 128], BF16)
    sc_in = nc.dram_tensor("sc_in", [ng2, 128], F32)
    sc_all = nc.dram_tensor("sc_all", [4 * ng2, 128], F32)
    rs_in = nc.dram_tensor("rs_in", [ng2 * 128, 128], BF16)
    rs_out = nc.dram_tensor("rs_out", [nrs, 128], BF16)

    all_group = [list(range(N_CORES))]
    pair_groups = [[2 * i, 2 * i + 1] for i in range(4)]
    attn_groups = [[0, 2, 4, 6], [1, 3, 5, 7]]

    with TileContext(nc) as tc:
        with (
            tc.tile_pool(name="const", bufs=1) as cpool,
            tc.tile_pool(name="sb", bufs=3) as sb,
            tc.tile_pool(name="sbg", bufs=2) as sbg,
            tc.tile_pool(name="sbeq", bufs=4) as sbeq,
            tc.tile_pool(name="psum", bufs=2, space="PSUM") as psum,
        ):
            def cload(src, c0, cols, tag, dt):
                t = cpool.tile([128, cols], dt, tag=tag)
                nc.sync.dma_start(out=t[:], in_=src[:, c0:c0 + cols])
                return t

            iota_t = cload(cst, 0, 128, "c_iota", F32)
            sel_t = cload(cst, 128, 4, "c_sel", F32)
            basep_t = cload(cst, 132, 1, "c_basep", F32)
            wm1_t = cload(wts, 0, 128, "c_wm1", BF16)
            wr1_t = cload(wts, 128, 128, "c_wr1", BF16)
            wm2_t = cload(wts, 256, 128, "c_wm2", BF16)
            wr2_t = cload(wts, 384, 128, "c_wr2", BF16)
            qs_t = cload(wts, 512, 128, "c_qs", BF16)
            ident_t = cload(wts, 640, 128, "c_ident", BF16)
            score_sb = cpool.tile([128, ng2], F32, tag="c_score")

            pools = (sb, sbg, psum, sbeq)

            nc.sync.dma_start(out=e_loc[:, :], in_=e_sh[:, :])
            nc.gpsimd.collective_compute(
                "AllGather", mybir.AluOpType.bypass,
                replica_groups=all_group,
                ins=[e_loc[:, :]], outs=[e_full[:, :]])

            # materialize x0perm = E[eids] in parity-permuted block order;
            # eip ships as 3 byte-sections (b0 + 256*b1 + 65536*hi)
            eoff = 4 * S1
            for c0 in range(0, tb, CH):
                w = min(CH, tb - c0)
                e0t = sbg.tile([128, w], U8, tag="e0t")
                nc.sync.dma_start(out=e0t[:],
                                  in_=g_all[:, eoff + c0:eoff + c0 + w])
                e1t = sbg.tile([128, w], U8, tag="e1t")
                nc.sync.dma_start(
                    out=e1t[:], in_=g_all[:, eoff + tb + c0:eoff + tb + c0 + w])
                ehit = sbg.tile([128, w], U8, tag="ehit")
                nc.sync.dma_start(
                    out=ehit[:],
                    in_=g_all[:, eoff + 2 * tb + c0:eoff + 2 * tb + c0 + w])
                elof = sbg.tile([128, w], F32, tag="elof")
                nc.vector.tensor_copy(out=elof[:], in_=e0t[:])
                e1f = sbg.tile([128, w], F32, tag="e1f")
                nc.vector.tensor_scalar(out=e1f[:], in0=e1t[:],
                                        scalar1=256.0, scalar2=None,
                                        op0=mybir.AluOpType.mult)
                nc.vector.tensor_tensor(out=elof[:], in0=elof[:], in1=e1f[:],
                                        op=mybir.AluOpType.add)
                ehif = sbg.tile([128, w], F32, tag="ehif")
                nc.vector.tensor_scalar(out=ehif[:], in0=ehit[:],
                                        scalar1=65536.0, scalar2=None,
                                        op0=mybir.AluOpType.mult)
                eipt = sbg.tile([128, w], I32, tag="eipt")
                nc.vector.tensor_tensor(out=eipt[:], in0=elof[:], in1=ehif[:],
                                        op=mybir.AluOpType.add)
                for j in range(w):
                    xt = sb.tile([128, 128], I8, tag="x0p_t")
                    nc.gpsimd.indirect_dma_start(
                        out=xt[:], out_offset=None, in_=e_full[:],
                        in_offset=bass.IndirectOffsetOnAxis(
                            ap=eipt[:, j:j + 1], axis=0))
                    nc.sync.dma_start(
                        out=x0p[(c0 + j) * 128:(c0 + j + 1) * 128, :],
                        in_=xt[:])

            _emit_layer(nc, tc, pools, x0p, True, 1, g_all, S1,
                        basep_t, wm1_t, wr1_t, ng1, nb1, iota_t, ident_t,
                        x1_half, nperm)

            nc.gpsimd.collective_compute(
                "AllGather", mybir.AluOpType.bypass,
                replica_groups=pair_groups,
                ins=[x1_half[:, :]], outs=[x1_full[:, :]])

            def score_hook(g, xn):
                t = sb.tile([128, 128], F32, tag="sc_tmp")
                nc.vector.tensor_tensor(out=t[:], in0=xn, in1=qs_t[:],
                                        op=mybir.AluOpType.mult)
                nc.vector.reduce_sum(out=score_sb[:, g:g + 1], in_=t[:],
                                     axis=mybir.AxisListType.X)

            # L2 reuses the first ng2 groups of the L1 grid verbatim
            _emit_layer(nc, tc, pools, x1_full, False, 1, g_all, S1,
                        basep_t, wm2_t, wr2_t, ng2, nb1, iota_t, ident_t,
                        x2b, ng2 * 128, hook=score_hook)

            nc.sync.dma_start(out=sc_in[:, :].rearrange("t p -> p t"),
                              in_=score_sb[:, :])
            nc.gpsimd.collective_compute(
                "AllGather", mybir.AluOpType.bypass,
                replica_groups=attn_groups,
                ins=[sc_in[:, :]], outs=[sc_all[:, :]])

            # softmax over 4 metapaths (elementwise across four [128,ng2] tiles)
            s_t = []
            for p in range(4):
                st = cpool.tile([128, ng2], F32, tag=f"s{p}")
                nc.sync.dma_start(
                    out=st[:],
                    in_=sc_all[p * ng2:(p + 1) * ng2, :].rearrange("t p -> p t"))
                s_t.append(st)
            m = cpool.tile([128, ng2], F32, tag="c_m")
            nc.vector.tensor_tensor(out=m[:], in0=s_t[0][:], in1=s_t[1][:],
                                    op=mybir.AluOpType.max)
            for p in (2, 3):
                nc.vector.tensor_tensor(out=m[:], in0=m[:], in1=s_t[p][:],
                                        op=mybir.AluOpType.max)
            e_t = []
            for p in range(4):
                dt_ = cpool.tile([128, ng2], F32, tag=f"d{p}")
                nc.vector.tensor_tensor(out=dt_[:], in0=s_t[p][:], in1=m[:],
                                        op=mybir.AluOpType.subtract)
                et = cpool.tile([128, ng2], F32, tag=f"e{p}")
                nc.scalar.activation(out=et[:], in_=dt_[:],
                                     func=mybir.ActivationFunctionType.Exp)
                e_t.append(et)
            z = cpool.tile([128, ng2], F32, tag="c_z")
            nc.vector.tensor_tensor(out=z[:], in0=e_t[0][:], in1=e_t[1][:],
                                    op=mybir.AluOpType.add)
            for p in (2, 3):
                nc.vector.tensor_tensor(out=z[:], in0=z[:], in1=e_t[p][:],
                                        op=mybir.AluOpType.add)
            rz = cpool.tile([128, ng2], F32, tag="c_rz")
            nc.vector.reciprocal(out=rz[:], in_=z[:])
            wown = cpool.tile([128, ng2], F32, tag="c_wown")
            acc = cpool.tile([128, ng2], F32, tag="c_acc")
            nc.vector.tensor_scalar(out=wown[:], in0=e_t[0][:],
                                    scalar1=sel_t[:, 0:1], scalar2=None,
                                    op0=mybir.AluOpType.mult)
            for p in (1, 2, 3):
                nc.vector.tensor_scalar(out=acc[:], in0=e_t[p][:],
                                        scalar1=sel_t[:, p:p + 1], scalar2=None,
                                        op0=mybir.AluOpType.mult)
                nc.vector.tensor_tensor(out=wown[:], in0=wown[:], in1=acc[:],
                                        op=mybir.AluOpType.add)
            nc.vector.tensor_tensor(out=wown[:], in0=wown[:], in1=rz[:],
                                    op=mybir.AluOpType.mult)

            # weighted partials, batched BF groups per DMA
            for g0 in range(0, ng2, BF):
                bw = min(BF, ng2 - g0)
                xt = sb.tile([128, bw * 128], BF16, tag="attn_x")
                nc.sync.dma_start(
                    out=xt[:].rearrange("p (a f) -> p a f", f=128),
                    in_=x2b[g0 * 128:(g0 + bw) * 128, :]
                    .rearrange("(a t) f -> t a f", t=128))
                wt = sb.tile([128, bw * 128], BF16, tag="attn_w")
                for j in range(bw):
                    nc.vector.tensor_scalar(
                        out=wt[:, j * 128:(j + 1) * 128],
                        in0=xt[:, j * 128:(j + 1) * 128],
                        scalar1=wown[:, g0 + j:g0 + j + 1], scalar2=None,
                        op0=mybir.AluOpType.mult)
                nc.sync.dma_start(
                    out=rs_in[g0 * 128:(g0 + bw) * 128, :]
                    .rearrange("(a t) f -> t a f", t=128),
                    in_=wt[:].rearrange("p (a f) -> p a f", f=128))

            nc.gpsimd.collective_compute(
                "ReduceScatter", mybir.AluOpType.add,
                replica_groups=attn_groups,
                ins=[rs_in[:, :]], outs=[rs_out[:, :]])

            # rs_out [nrs,128] -> out_part, bounced through SBUF
            nblk = nrs // 128
            fin = cpool.tile([128, nblk * 128], BF16, tag="c_fin")
            nc.sync.dma_start(
                out=fin[:].rearrange("p (a f) -> p a f", f=128),
                in_=rs_out[:, :].rearrange("(a t) f -> t a f", t=128))
            nc.sync.dma_start(
                out=out_part[:, :].rearrange("(a t) f -> t a f", t=128),
                in_=fin[:].rearrange("p (a f) -> p a f", f=128))
    return nc


# ----------------------------------------------------------------- kernel()

def kernel(E, metapath_emb, W_root, W_rel, b, Wq, bq, edge_index, eids,
           nreg=50000, trace=False, debug=False):
    bf16 = _np_bf16()
    P = edge_index.shape[0]
    n = eids.shape[1]
    d = E.shape[1]
    etab = E.shape[0]
    scale = np.float32(1.0 / math.sqrt(d))
    assert P == 4 and d == 128 and n == 2 * nreg

    E = np.asarray(E, np.float32)
    edge_index = np.asarray(edge_index)
    eids = np.asarray(eids).astype(np.int32)
    assert not np.any(np.asarray(b)), "nonzero bias not supported"

    # keep only E rows some eids references, then int8-quantize (dequant
    # scale folds into the L1 weights)
    used = np.zeros(etab, bool)
    used[eids.ravel()] = True
    remap = np.cumsum(used, dtype=np.int64) - 1
    eids = remap[eids].astype(np.int32)
    e_used = E[used]
    nu = e_used.shape[0]
    nsh = (nu + N_CORES - 1) // N_CORES
    etab = nsh * N_CORES
    e_scale = np.float32(max(float(np.abs(e_used).max()), 1e-30) / 127.0)
    e_q = np.zeros((etab, 128), np.int8)
    e_q[:nu] = np.clip(np.rint(e_used / e_scale), -127, 127)

    query = (np.asarray(metapath_emb, np.float32) @ np.asarray(Wq, np.float32)
             + np.asarray(bq, np.float32))
    query_scaled = query * scale

    tbh = math.ceil(n / 256)     # 128-blocks per parity side
    ng1 = tbh                    # L1 groups per core (one parity side)
    ng2 = math.ceil(nreg / 256)  # L2 groups per core
    assert ng2 <= ng1
    nperm = ng1 * 128
    tb = 2 * ng1

    # perm(j): parity-split node order
    def perm(j):
        blk = j >> 7
        pos = j & 127
        return (blk & 1) * nperm + (blk >> 1) * 128 + pos

    # per-metapath: degree, edges split by dst-block parity, sorted by dst
    metas = []
    for i in range(P):
        src = edge_index[i, 0].astype(np.int64)
        dst = edge_index[i, 1].astype(np.int64)
        deg = np.bincount(dst, minlength=n)
        degc = np.maximum(deg, 1).astype(np.int64)
        order = np.argsort(dst, kind="stable")
        ssrc, sdst = src[order], dst[order]
        side = (sdst >> 7) & 1
        per_h = []
        for h in (0, 1):
            m = side == h
            es, ed = ssrc[m], sdst[m]
            per_h.append((es, ed))
        metas.append((degc, per_h))

    # nb1 = max blocks over every (core, group)
    nb1 = 1
    counts = []
    for i in range(P):
        for h in (0, 1):
            es, ed = metas[i][1][h]
            gidx = (ed >> 8)                      # (blk>>1): group id
            cnt = np.bincount(gidx, minlength=ng1)
            counts.append(cnt)
            nb1 = max(nb1, math.ceil(cnt.max() / 128))
    S1 = nb1 * ng1

    iota = np.tile(np.arange(128, dtype=np.float32), (128, 1))
    ident = np.eye(128, dtype=np.float32)
    pos_col = np.arange(128, dtype=np.int64)[:, None]

    in_maps = []
    for c in range(N_CORES):
        i, h = c // 2, c % 2
        degc, per_h = metas[i]
        es, ed = per_h[h]
        gidx = (ed >> 8).astype(np.int64)
        dl = ed & 127
        lo16, pk8, dl8 = _build_grids(perm(es), gidx, dl, degc[ed], ng1, nb1)
        # x0perm materialization indices: block c -> global block
        cb = np.arange(tb)[None, :]
        gblk = np.where(cb < ng1, 2 * cb, 2 * (cb - ng1) + 1)
        jj = np.minimum(gblk * 128 + pos_col, n - 1)
        eip = eids[i][jj].astype(np.int64)
        eip_b0 = (eip & 0xFF).astype(np.uint8)
        eip_b1 = ((eip >> 8) & 0xFF).astype(np.uint8)
        eip_hi = (eip >> 16).astype(np.uint8)
        selm = np.zeros((128, 4), np.float32)
        selm[:, i] = 1.0
        basep = (h * nperm + np.arange(128)).astype(np.float32)[:, None]
        wmat = np.concatenate([
            (np.asarray(W_rel[i, 0], np.float32) * e_scale),
            (np.asarray(W_root[i, 0], np.float32) * e_scale),
            np.asarray(W_rel[i, 1], np.float32),
            np.asarray(W_root[i, 1], np.float32),
            np.tile(query_scaled[i], (128, 1)).astype(np.float32),
            ident,
        ], axis=1).astype(bf16)
        in_maps.append(dict(
            e_sh=np.ascontiguousarray(e_q[c * nsh:(c + 1) * nsh]),
            g_all=np.concatenate(
                [(lo16 & 0xFF).astype(np.uint8),
                 (lo16 >> 8).astype(np.uint8),
                 pk8, dl8, eip_b0, eip_b1, eip_hi], axis=1),
            cst=np.concatenate([iota, selm, basep],
                               axis=1).astype(np.float32),
            wts=wmat,
        ))

    nc = build_program(etab, ng1, nb1, ng2)
    nc.compile()
    kernel.last_nc = nc
    kernel.last_in_maps = in_maps
    res = run_bass_kernel_spmd(nc, in_maps, core_ids=list(range(N_CORES)),
                               trace=trace)

    # interleave parity sides back to node order
    a_rows = np.concatenate(
        [res.results[c]["out_part"].astype(np.float32) for c in (0, 2, 4, 6)],
        axis=0).reshape(ng2, 128, 128)
    b_rows = np.concatenate(
        [res.results[c]["out_part"].astype(np.float32) for c in (1, 3, 5, 7)],
        axis=0).reshape(ng2, 128, 128)
    full = np.empty((2 * ng2, 128, 128), np.float32)
    full[0::2] = a_rows
    full[1::2] = b_rows
    out = full.reshape(-1, 128)[:nreg].astype(np.float32)
    kernel.last_results = res
    return out
